# revision 43
# baseline (speedup 1.0000x reference)
"""Causal self-attention Trainium2 kernel (B=2, T=2048, D=1024, 16 heads).

Sharding: 8 cores = 2 batches x 4 head-groups (4 heads each).
Per core: column-parallel qkv, local attention, row-parallel proj producing a
partial output; host sums the 4 partials per batch and adds proj bias.
"""

import json
import math
import os

import numpy as np
import ml_dtypes

import bass_rust
import concourse.bass as bass
import concourse.bass2jax as bass2jax
import concourse.bass_utils as bass_utils
import concourse.mybir as mybir
import concourse.tile as tile
from concourse.tile import TileContext
from concourse.vector_clock import ScopedClock
from concourse.bass_utils import run_bass_kernel_spmd

BF16 = mybir.dt.bfloat16
F32 = mybir.dt.float32
FP8 = mybir.dt.float8e4
NP_BF16 = ml_dtypes.bfloat16
NP_FP8 = ml_dtypes.float8_e4m3

D_MODEL = 1024
N_HEAD = 16
D_HEAD = 64
B = 2
T = 2048
G = 4                    # head-groups (tensor parallel)
HPC = N_HEAD // G        # heads per core = 4
QKC = 2 * HPC * D_HEAD   # q+k rows per core = 512
VC = HPC * D_HEAD        # v cols per core = 256
TCH = 512                # t-chunk (matmul moving free dim)
NTJ = T // TCH           # 4 t-chunks
NSI = T // 128           # 16 s-blocks
KC = D_MODEL // 128      # 8 contraction chunks over d_model
SCALE = 1.0 / math.sqrt(D_HEAD)
W_SCALE = 32.0           # host pre-scale on qkv_w for the fp8 hi/lo split
                         # (keeps wlo above the e4m3 subnormal flush zone)


def _split_multi_waits(bir_json: bytes) -> bytes:
    """The walrus build in this container accepts at most one sync-wait
    command per instruction. Split instructions with N>1 waits into N-1
    preceding single-wait NoOps on the same engine (sequential waits AND
    together, so semantics are unchanged)."""
    bir = json.loads(bir_json)
    ctr = 0
    changed = False
    for func in bir.get("functions", []):
        for blk in func.get("blocks", []):
            out = []
            for inst in blk.get("instructions", []):
                si = inst.get("sync_info")
                waits = (si or {}).get("on_wait") or []
                if len(waits) > 1:
                    changed = True
                    for w in waits[:-1]:
                        ctr += 1
                        out.append(
                            {
                                "debug": inst.get("debug", 0),
                                "engine": inst.get("engine"),
                                "ins": [],
                                "name": f"WSPLIT-{ctr}",
                                "opcode": "NoOp",
                                "outs": [],
                                "sync_info": {"on_update": [], "on_wait": [w]},
                            }
                        )
                    si["on_wait"] = [waits[-1]]
                out.append(inst)
            blk["instructions"] = out
    if not changed:
        return bir_json
    return json.dumps(bir).encode()


_orig_compile_bir_kernel = bass_utils.compile_bir_kernel.__wrapped__ if hasattr(
    bass_utils.compile_bir_kernel, "__wrapped__"
) else bass_utils.compile_bir_kernel


def _patched_compile_bir_kernel(bir_json, tmpdir, neff_name="file.neff"):
    return _orig_compile_bir_kernel(_split_multi_waits(bir_json), tmpdir, neff_name)


def _patch_drain():
    """The walrus build in this container rejects >1 sync-wait command per
    instruction. Patch the compile path to split waits, and the TileContext
    terminal drain to emit single-wait SP nops."""
    if getattr(TileContext, "_drain_patched", False):
        return
    bass_utils.compile_bir_kernel = _patched_compile_bir_kernel
    bass2jax.compile_bir_kernel = _patched_compile_bir_kernel

    def _drain_and_barrier(self, tick_clock, wait_clock):
        nc = self.nc
        probe = nc.sync.nop()
        wait_clock.add_sem_waits(
            probe.ins, ScopedClock({None: tick_clock.global_clock})
        )
        si = probe.ins.sync_info
        waits = list(si.on_wait) if si is not None else []
        if si is not None:
            si.on_wait = waits[:1]
            probe.ins.sync_info = si
        for w in waits[1:]:
            n = nc.sync.nop()
            n.ins.sync_info = bass_rust.SyncInfo(on_wait=[w], on_update=[])
        nc.sync.drain()
        nc.all_engine_barrier()
        popped = nc._tile_sem_poison_stack.pop()
        assert popped is self._sem_poison
        nc.clear_and_free_semaphores(list(self.sems.allocated().values()))
        nc.all_engine_barrier()

    TileContext._drain_and_barrier = _drain_and_barrier
    TileContext._drain_patched = True


def _build(mask_mode: str, has_qkv_bias: bool, head_pair=None, si_pair=None):
    """mask_mode: 'causal' | 'none' | 'generic'"""
    if head_pair is None:
        head_pair = os.environ.get("K_HEAD_PAIR", "0") == "1"
    if si_pair is None:
        si_pair = os.environ.get("K_SI_PAIR", "1") == "1"
    interleave = os.environ.get("K_INTERLEAVE", "0") == "1"
    qkv_dr = os.environ.get("K_QKV_DR", "1") == "1"
    proj_dr = qkv_dr and os.environ.get("K_PROJ_DR", "1") == "1"
    copy_spread = os.environ.get("K_COPY_SPREAD", "0") == "1"
    exp_scale = SCALE / (W_SCALE * W_SCALE) if qkv_dr else SCALE
    _patch_drain()
    nc = bass.Bass()

    if qkv_dr:
        xhi_d = nc.dram_tensor("xhi", [D_MODEL, T], FP8, kind="ExternalInput")
        xlo_d = nc.dram_tensor("xlo", [D_MODEL, T], FP8, kind="ExternalInput")
        whi_d = nc.dram_tensor(
            "whi", [D_MODEL, QKC + VC], FP8, kind="ExternalInput"
        )
        wlo_d = nc.dram_tensor(
            "wlo", [D_MODEL, QKC + VC], FP8, kind="ExternalInput"
        )
    else:
        xT = nc.dram_tensor("xT", [D_MODEL, T], BF16, kind="ExternalInput")
        wqkv = nc.dram_tensor(
            "wqkv", [D_MODEL, QKC + VC], BF16, kind="ExternalInput"
        )
    wproj = nc.dram_tensor("wproj", [VC, D_MODEL], BF16, kind="ExternalInput")
    if proj_dr:
        wphi_d = nc.dram_tensor("wphi", [VC, D_MODEL], FP8, kind="ExternalInput")
        wplo_d = nc.dram_tensor("wplo", [VC, D_MODEL], FP8, kind="ExternalInput")
    if mask_mode == "causal":
        tri_d = nc.dram_tensor("tri", [128, 128], BF16, kind="ExternalInput")
    if mask_mode == "generic":
        maskT_d = nc.dram_tensor("maskT", [T, T], BF16, kind="ExternalInput")
    if has_qkv_bias:
        bqk_d = nc.dram_tensor("bqk", [QKC], F32, kind="ExternalInput")
        bv_d = nc.dram_tensor("bv", [VC], F32, kind="ExternalInput")
    out_dt = BF16 if os.environ.get("K_OUT_BF16", "1") == "1" else F32
    out_d = nc.dram_tensor("out", [T, D_MODEL], out_dt, kind="ExternalOutput")

    with TileContext(nc) as tc:
        with (
            tc.tile_pool(name="consts", bufs=1) as consts,
            tc.tile_pool(name="qkp", bufs=1) as qkp,
            tc.tile_pool(name="vp", bufs=1) as vp,
            tc.tile_pool(name="pp", bufs=int(os.environ.get("K_PP_BUFS", "2"))) as pp,
            tc.tile_pool(name="p3p", bufs=1) as p3p,
            tc.tile_pool(name="ap_", bufs=int(os.environ.get("K_AP_BUFS", "3"))) as ap_,
            tc.tile_pool(name="rp", bufs=int(os.environ.get("K_RP_BUFS", "2"))) as rp,
            tc.tile_pool(name="op_", bufs=int(os.environ.get("K_OSB_BUFS", "4"))) as op_,
            tc.tile_pool(name="dram_p", bufs=2, space="DRAM") as dram_p,
            tc.tile_pool(name="sa_ps", bufs=int(os.environ.get("K_SA_BUFS", str(4 // (2 if head_pair else 1) // (2 if si_pair else 1)))), space="PSUM") as sa_ps_pool,
            tc.tile_pool(name="sb_ps", bufs=(2 // (2 if si_pair else 1)), space="PSUM") as sb_ps_pool,
            tc.tile_pool(name="u_ps", bufs=int(os.environ.get("K_U_BUFS", "2")), space="PSUM") as u_ps_pool,
            tc.tile_pool(name="o_ps", bufs=int(os.environ.get("K_O_BUFS", "2")), space="PSUM") as o_ps_pool,
        ):
            # ---- load constants ----
            eng_map = {
                "s": nc.sync, "g": nc.gpsimd, "a": nc.scalar, "v": nc.vector,
            }
            dma_engs = [
                eng_map[ch] for ch in os.environ.get("K_DMA_SET", "sg")
            ]
            n_dma_eng = len(dma_engs)
            NQ = int(os.environ.get("K_XQ", "4"))
            KC2 = KC // 2  # 256-wide contraction chunks for DoubleRow
            if qkv_dr:
                # d = c2*256 + i*128 + p; hi/lo fp8 splits prepared on host
                xhi_r = xhi_d.rearrange("(c2 i p) t -> p c2 i t", p=128, i=2)
                xlo_r = xlo_d.rearrange("(c2 i p) t -> p c2 i t", p=128, i=2)
                whi_r = whi_d.rearrange("(c2 i p) n -> p c2 i n", p=128, i=2)
                wlo_r = wlo_d.rearrange("(c2 i p) n -> p c2 i n", p=128, i=2)
                XH = consts.tile([128, KC2, 2, T], FP8)
                XL = consts.tile([128, KC2, 2, T], FP8)
                WH = consts.tile([128, KC2, 2, QKC + VC], FP8)
                WL = consts.tile([128, KC2, 2, QKC + VC], FP8)
                # c2-major startup order: chunk c's W + X(q0) land together so
                # the first qkv matmuls start ~1.5us in and stream per-chunk
                ei = 0

                def dma(out, in_):
                    nonlocal ei
                    dma_engs[ei % n_dma_eng].dma_start(out=out, in_=in_)
                    ei += 1

                q0 = T // NQ
                for c in range(KC2):
                    dma(WH[:, c], whi_r[:, c])
                    dma(XH[:, c, :, 0:q0], xhi_r[:, c, :, 0:q0])
                    dma(WL[:, c], wlo_r[:, c])
                    dma(XL[:, c, :, 0:q0], xlo_r[:, c, :, 0:q0])
                for q in range(1, NQ):
                    lo, hi = q * (T // NQ), (q + 1) * (T // NQ)
                    for c in range(KC2):
                        dma(XH[:, c, :, lo:hi], xhi_r[:, c, :, lo:hi])
                        dma(XL[:, c, :, lo:hi], xlo_r[:, c, :, lo:hi])
            else:
                xT_r = xT.rearrange("(c p) t -> p c t", p=128)
                X = consts.tile([128, KC, T], BF16)
                wqkv_r = wqkv.rearrange("(c p) n -> p c n", p=128)
                W = consts.tile([128, KC, QKC + VC], BF16)
                for q in range(NQ):
                    lo, hi = q * (T // NQ), (q + 1) * (T // NQ)
                    for c in range(KC):
                        if q == 0:
                            dma_engs[c % n_dma_eng].dma_start(
                                out=W[:, c], in_=wqkv_r[:, c]
                            )
                        dma_engs[(q * KC + c + 1) % n_dma_eng].dma_start(
                            out=X[:, c, lo:hi], in_=xT_r[:, c, lo:hi]
                        )
            if mask_mode == "causal":
                tri = consts.tile([128, 128], BF16)
                nc.scalar.dma_start(out=tri, in_=tri_d[:, :])
            WP = consts.tile([128, VC // 128, D_MODEL], BF16)
            nc.sync.dma_start(out=WP, in_=wproj.rearrange("(c p) n -> p c n", p=128))
            if proj_dr:
                WPH = consts.tile([128, VC // 128, D_MODEL], FP8)
                WPL = consts.tile([128, VC // 128, D_MODEL], FP8)
                nc.sync.dma_start(
                    out=WPH, in_=wphi_d.rearrange("(c p) n -> p c n", p=128)
                )
                nc.sync.dma_start(
                    out=WPL, in_=wplo_d.rearrange("(c p) n -> p c n", p=128)
                )
            if mask_mode == "generic":
                MT = consts.tile([128, NSI, T], BF16)
                nc.sync.dma_start(
                    out=MT, in_=maskT_d.rearrange("(si p) t -> p si t", p=128)
                )
            if has_qkv_bias:
                bqk = consts.tile([128, QKC // 128], F32)
                nc.sync.dma_start(
                    out=bqk, in_=bqk_d.rearrange("(m p) -> p m", p=128)
                )
                bv = consts.tile([128, VC // 128], F32)
                nc.sync.dma_start(out=bv, in_=bv_d.rearrange("(m p) -> p m", p=128))

            # V tile (natural layout). Each head gets 64 ones-columns
            # appended so the PV matmul (M=128, same pass cost as M=65)
            # emits the softmax sums replicated on partitions 64..127 --
            # the reciprocal+normalize then needs no partition broadcast.
            V = vp.tile([128, NSI, HPC, 2 * D_HEAD], BF16)
            nc.vector.memset(V[:, :, :, D_HEAD : 2 * D_HEAD], 1.0)
            QK = qkp.tile([128, QKC // 128, T], BF16)

            DRM = mybir.MatmulPerfMode.DoubleRow

            qkv_pair = os.environ.get("K_QKV_PAIR", "0") == "1"

            def _qk_copy(m, j, qk_ps):
                if has_qkv_bias:
                    nc.scalar.activation(
                        out=QK[:, m, j * TCH : (j + 1) * TCH],
                        in_=qk_ps,
                        func=mybir.ActivationFunctionType.Identity,
                        bias=bqk[:, m : m + 1],
                    )
                elif copy_spread and m % 2 == 1:
                    nc.scalar.activation(
                        out=QK[:, m, j * TCH : (j + 1) * TCH],
                        in_=qk_ps,
                        func=mybir.ActivationFunctionType.Identity,
                    )
                else:
                    nc.vector.tensor_copy(
                        out=QK[:, m, j * TCH : (j + 1) * TCH], in_=qk_ps
                    )

            def _v_copy(i, v_ps):
                if copy_spread and i % 2 == 1:
                    nc.scalar.activation(
                        out=V[:, i, :, 0:D_HEAD],
                        in_=v_ps[:, :VC].rearrange("p (h d) -> p h d", h=HPC),
                        func=mybir.ActivationFunctionType.Identity,
                    )
                else:
                    nc.vector.tensor_copy(
                        out=V[:, i, :, 0:D_HEAD],
                        in_=v_ps[:, :VC].rearrange("p (h d) -> p h d", h=HPC),
                    )

            def emit_qkv_chunk(j):
                # Q^T / K^T chunk j: [qkrow, t] = sum_c W[c, qkrow] X^T[c, t]
                QK_TERMS = [(WH, XH), (WL, XH), (WH, XL)] if qkv_dr else None
                V_TERMS = [(XH, WH), (XH, WL), (XL, WH)] if qkv_dr else None
                if qkv_dr and qkv_pair:
                    # pairwise c2-major emission: two live psum accumulations
                    # sweep the contraction chunks together, so the in-order
                    # PE stream never head-blocks on a late X/W chunk
                    ts = slice(j * TCH, (j + 1) * TCH)
                    for m0 in (0, 2):
                        pss = {
                            m: o_ps_pool.tile(
                                [128, TCH], F32, tag="ops", name=f"qk_ps{m}"
                            )
                            for m in (m0, m0 + 1)
                        }
                        for c in range(KC2):
                            for m in (m0, m0 + 1):
                                ms = slice(m * 128, (m + 1) * 128)
                                for ti, (Wt, Xt) in enumerate(QK_TERMS):
                                    nc.tensor.matmul(
                                        pss[m],
                                        Wt[:, c, :, ms],
                                        Xt[:, c, :, ts],
                                        start=(c == 0 and ti == 0),
                                        stop=(c == KC2 - 1 and ti == 2),
                                        perf_mode=DRM,
                                    )
                        for m in (m0, m0 + 1):
                            _qk_copy(m, j, pss[m])
                    vs = slice(QKC, QKC + VC)
                    for i0 in (4 * j, 4 * j + 2):
                        pss = {
                            i: o_ps_pool.tile(
                                [128, TCH], F32, tag="ops", name=f"v_ps{i}"
                            )
                            for i in (i0, i0 + 1)
                        }
                        for c in range(KC2):
                            for i in (i0, i0 + 1):
                                isl = slice(i * 128, (i + 1) * 128)
                                for ti, (Xt, Wt) in enumerate(V_TERMS):
                                    nc.tensor.matmul(
                                        pss[i][:, :VC],
                                        Xt[:, c, :, isl],
                                        Wt[:, c, :, vs],
                                        start=(c == 0 and ti == 0),
                                        stop=(c == KC2 - 1 and ti == 2),
                                        perf_mode=DRM,
                                    )
                        for i in (i0, i0 + 1):
                            _v_copy(i, pss[i])
                    return
                term_major = os.environ.get("K_TERM_MAJOR", "0") == "1"
                for m in range(QKC // 128):
                    qk_ps = o_ps_pool.tile([128, TCH], F32, tag="ops")
                    if qkv_dr:
                        ms = slice(m * 128, (m + 1) * 128)
                        ts = slice(j * TCH, (j + 1) * TCH)
                        # (Whi+Wlo)@(Xhi+Xlo) dropping the lo*lo term
                        n_mm = KC2 * len(QK_TERMS)
                        if term_major:
                            order = [
                                (c, wx) for wx in QK_TERMS for c in range(KC2)
                            ]
                        else:
                            order = [
                                (c, wx) for c in range(KC2) for wx in QK_TERMS
                            ]
                        for k, (c, (Wt, Xt)) in enumerate(order):
                            nc.tensor.matmul(
                                qk_ps,
                                Wt[:, c, :, ms],
                                Xt[:, c, :, ts],
                                start=(k == 0),
                                stop=(k == n_mm - 1),
                                perf_mode=DRM,
                            )
                    else:
                        for c in range(KC):
                            nc.tensor.matmul(
                                qk_ps,
                                W[:, c, m * 128 : (m + 1) * 128],
                                X[:, c, j * TCH : (j + 1) * TCH],
                                start=(c == 0),
                                stop=(c == KC - 1),
                            )
                    _qk_copy(m, j, qk_ps)
                # V rows for this chunk
                for i in range(4 * j, 4 * j + 4):
                    v_ps = o_ps_pool.tile([128, TCH], F32, tag="ops")
                    if qkv_dr:
                        isl = slice(i * 128, (i + 1) * 128)
                        vs = slice(QKC, QKC + VC)
                        n_mm = KC2 * len(V_TERMS)
                        k = 0
                        for c in range(KC2):
                            for (Xt, Wt) in V_TERMS:
                                nc.tensor.matmul(
                                    v_ps[:, :VC],
                                    Xt[:, c, :, isl],
                                    Wt[:, c, :, vs],
                                    start=(k == 0),
                                    stop=(k == n_mm - 1),
                                    perf_mode=DRM,
                                )
                                k += 1
                    else:
                        for c in range(KC):
                            nc.tensor.matmul(
                                v_ps[:, :VC],
                                X[:, c, i * 128 : (i + 1) * 128],
                                W[:, c, QKC : QKC + VC],
                                start=(c == 0),
                                stop=(c == KC - 1),
                            )
                    _v_copy(i, v_ps)

            pre3 = (
                os.environ.get("K_PRE3", "1") == "1" and mask_mode == "causal"
            )
            P3 = {}

            def emit_attention(tj, phase="full"):
                # phase: "full" | "scores" (S/exp/mask only, into P3 tiles)
                #        | "pv" (PV/norm/proj consuming P3 tiles)
                is_tail = phase == "pv" or tj == NTJ - 1
                use_pdr = proj_dr and not is_tail
                if phase != "scores":
                    A = ap_.tile([128, VC // 128, TCH], BF16)
                    if use_pdr:
                        A8H = ap_.tile(
                            [128, VC // 128, TCH], FP8, tag="a8h", name="A8H"
                        )
                        A8L = ap_.tile(
                            [128, VC // 128, TCH], FP8, tag="a8l", name="A8L"
                        )
                n_si = NSI if mask_mode != "causal" else 4 * tj + 4
                p_slices = 12 if pre3 else NSI
                HGRP = 2 if head_pair else 1
                SGRP = 2 if si_pair else 1
                def emit_head_group(hp):
                    heads = tuple(HGRP * hp + u for u in range(HGRP))
                    Us = {}
                    Ps = {}
                    for h in heads:
                        if phase != "scores":
                            Us[h] = u_ps_pool.tile(
                                [2 * D_HEAD, TCH], F32, tag="u", name="U"
                            )
                        if phase == "scores":
                            P3[h] = p3p.tile(
                                [128, NSI, TCH], BF16, tag=f"p3h{h}", name="P3"
                            )
                            Ps[h] = P3[h]
                        elif phase == "pv":
                            Ps[h] = P3[h]
                        else:
                            Ps[h] = pp.tile(
                                [128, p_slices, TCH], BF16, tag="p", name="P"
                            )
                    sp_order = list(range(n_si // SGRP))
                    if os.environ.get("K_SP_REV", "0") == "1":
                        sp_order = sp_order[::-1]
                    first_sp = sp_order[0]
                    last_sp = sp_order[-1]
                    pv_defer = os.environ.get("K_PV_DEFER", "0") == "1"
                    prev_sp = [None]
                    for sp in sp_order:
                        spair = tuple(SGRP * sp + u for u in range(SGRP))
                        s_tiles = {}
                        for hi, h in enumerate(heads):
                            if phase == "pv":
                                break
                            pool = sa_ps_pool if hi == 0 else sb_ps_pool
                            s_ps = pool.tile([128, SGRP, TCH], F32, tag="s", name="s_ps")
                            s_tiles[h] = s_ps
                            pb = 64 * (h % 2)
                            qm = h // 2
                            km = 2 + h // 2
                            for u, si in enumerate(spair):
                                if mask_mode == "causal" and si >= 4 * tj:
                                    coff = 128 * (si - 4 * tj)
                                else:
                                    coff = 0
                                nc.tensor.matmul(
                                    s_ps[:, u, coff:TCH],
                                    QK[pb : pb + 64, km, si * 128 : (si + 1) * 128],
                                    QK[
                                        pb : pb + 64,
                                        qm,
                                        tj * TCH + coff : (tj + 1) * TCH,
                                    ],
                                    start=True,
                                    stop=True,
                                )
                        exp_split = (
                            os.environ.get("K_EXP_SPLIT", "0") == "1"
                            or tj >= int(os.environ.get("K_EXP_SPLIT_TJ", "99"))
                        )
                        for h in heads:
                            if phase == "pv":
                                break
                            # exp over the si-pair (prefixes of diagonal
                            # blocks hold garbage; never read back)
                            if exp_split:
                                for u in range(SGRP):
                                    nc.scalar.activation(
                                        out=Ps[h][:, SGRP * sp + u, :],
                                        in_=s_tiles[h][:, u, :],
                                        func=mybir.ActivationFunctionType.Exp,
                                        scale=exp_scale,
                                    )
                            else:
                                if mask_mode == "causal" and spair[0] >= 4 * tj:
                                    pcoff = 128 * (spair[0] - 4 * tj)
                                else:
                                    pcoff = 0
                                nc.scalar.activation(
                                    out=Ps[h][:, SGRP * sp : SGRP * sp + SGRP, pcoff:],
                                    in_=s_tiles[h][:, :, pcoff:],
                                    func=mybir.ActivationFunctionType.Exp,
                                    scale=exp_scale,
                                )
                            for si in spair:
                                if mask_mode == "causal" and si >= 4 * tj:
                                    coff = 128 * (si - 4 * tj)
                                    nc.vector.tensor_tensor(
                                        Ps[h][:, si, coff : coff + 128],
                                        Ps[h][:, si, coff : coff + 128],
                                        tri,
                                        mybir.AluOpType.mult,
                                    )
                            if mask_mode == "generic":
                                for si in spair:
                                    nc.vector.tensor_tensor(
                                        Ps[h][:, si, :],
                                        Ps[h][:, si, :],
                                        MT[:, si, tj * TCH : (tj + 1) * TCH],
                                        mybir.AluOpType.mult,
                                    )
                        def emit_pv(sp_):
                            spair_ = tuple(
                                SGRP * sp_ + u for u in range(SGRP)
                            )
                            for h in heads:
                                if phase == "scores":
                                    break
                                for si in spair_:
                                    if mask_mode == "causal" and si >= 4 * tj:
                                        coff = 128 * (si - 4 * tj)
                                    else:
                                        coff = 0
                                    nc.tensor.matmul(
                                        Us[h][:, coff:TCH],
                                        V[:, si, h, :],
                                        Ps[h][:, si, coff:TCH],
                                        start=(
                                            sp_ == first_sp
                                            and si == spair_[0]
                                        ),
                                        stop=(
                                            sp_ == last_sp
                                            and si == spair_[-1]
                                        ),
                                        skip_group_check=True,
                                    )

                        if pv_defer and phase != "scores":
                            # software-pipeline: PV(sp-1) lands after S(sp)
                            # + exp(sp) so the exp latency is covered by PE
                            # work in program order
                            if sp != first_sp:
                                emit_pv(prev_sp[0])
                            prev_sp[0] = sp
                            if sp == last_sp:
                                emit_pv(sp)
                        else:
                            emit_pv(sp)
                    for h in heads:
                        if phase == "scores":
                            break
                        # normalize: sums sit replicated on partitions
                        # 64..127 of U; reciprocal them straight to SBUF
                        pb = 64 * (h % 2)
                        Rb_sb = rp.tile([64, TCH], F32, tag="rbsb")
                        a_slice = A[pb : pb + 64, h // 2, :]
                        # split the last chunk's norm into t-halves so proj
                        # can start on the first half sooner
                        n_nsp = 2 if is_tail else 1
                        hs = TCH // n_nsp
                        for u in range(n_nsp):
                            ts_ = slice(u * hs, (u + 1) * hs)
                            nc.vector.reciprocal(
                                Rb_sb[:, ts_], Us[h][D_HEAD : 2 * D_HEAD, ts_]
                            )
                            nc.vector.tensor_tensor(
                                a_slice[:, ts_],
                                Us[h][0:D_HEAD, ts_],
                                Rb_sb[:, ts_],
                                mybir.AluOpType.mult,
                            )
                        if has_qkv_bias:
                            nc.scalar.activation(
                                out=a_slice,
                                in_=a_slice,
                                func=mybir.ActivationFunctionType.Identity,
                                bias=bv[pb : pb + 64, h // 2 : h // 2 + 1],
                            )
                        if use_pdr:
                            # hi/lo fp8 split of A off the critical engines
                            ah = A8H[pb : pb + 64, h // 2, :]
                            al = A8L[pb : pb + 64, h // 2, :]
                            a8e = (
                                nc.vector
                                if os.environ.get("K_A8_ENG", "g") == "v"
                                else nc.gpsimd
                            )
                            a8e.tensor_copy(out=ah, in_=a_slice)
                            a8e.tensor_tensor(
                                al, a_slice, ah, mybir.AluOpType.subtract
                            )

                head_ilv = os.environ.get("K_HEAD_ILV", "0") == "1"
                for hp in range(HPC // HGRP):
                    if head_ilv and hp % 2 == 1:
                        off = tc.cur_priority - pair_base
                        with tc.high_priority(offset=off):
                            emit_head_group(hp)
                    else:
                        pair_base = tc.cur_priority
                        emit_head_group(hp)
                if phase == "scores":
                    return
                # proj for this t-chunk: out[t, n] = sum_c A^T[c, t] * WP[c, n]
                tail_tj = phase == "pv" or tj == NTJ - 1
                tail_ups = os.environ.get("K_TAIL_UPS", "0") == "1"
                for tb in range(TCH // 128):
                    o_sb = op_.tile([128, D_MODEL], out_dt)
                    for n in range(D_MODEL // TCH):
                        # tail: alternate the proj psum between the o and u
                        # pools (u is idle by now) to deepen the ring
                        if tail_tj and tail_ups and (2 * tb + n) % 2 == 1:
                            o_ps = u_ps_pool.tile(
                                [128, TCH], F32, tag="u", name="o_ps_u"
                            )
                        else:
                            o_ps = o_ps_pool.tile([128, TCH], F32, tag="ops")
                        if use_pdr:
                            tbs = slice(tb * 128, (tb + 1) * 128)
                            nsl = slice(n * TCH, (n + 1) * TCH)
                            pterms = [(A8H, WPH), (A8L, WPH), (A8H, WPL)]
                            for k, (At, Wt) in enumerate(pterms):
                                nc.tensor.matmul(
                                    o_ps,
                                    At[:, :, tbs],
                                    Wt[:, :, nsl],
                                    start=(k == 0),
                                    stop=(k == len(pterms) - 1),
                                    perf_mode=DRM,
                                )
                        else:
                            for c in range(VC // 128):
                                nc.tensor.matmul(
                                    o_ps,
                                    A[:, c, tb * 128 : (tb + 1) * 128],
                                    WP[:, c, n * TCH : (n + 1) * TCH],
                                    start=(c == 0),
                                    stop=(c == VC // 128 - 1),
                                )
                        # tail chunk: spread psum->sbuf copies over DVE+Act
                        # so the drain isn't DVE-copy paced
                        tmode = int(os.environ.get("K_TAIL_MODE", "0"))
                        dq = nc.sync
                        if tail_tj and tmode in (1, 2):
                            h0 = slice(n * TCH, n * TCH + TCH // 2)
                            h1 = slice(n * TCH + TCH // 2, (n + 1) * TCH)
                            nc.vector.tensor_copy(
                                out=o_sb[:, h0], in_=o_ps[:, 0 : TCH // 2]
                            )
                            nc.scalar.activation(
                                out=o_sb[:, h1],
                                in_=o_ps[:, TCH // 2 : TCH],
                                func=mybir.ActivationFunctionType.Identity,
                            )
                            if tmode == 1:
                                dq = (
                                    nc.sync
                                    if (2 * tb + n) % 2 == 0
                                    else nc.gpsimd
                                )
                        elif tail_tj and tmode in (0, 3):
                            if (2 * tb + n) % 2 == 1:
                                nc.scalar.activation(
                                    out=o_sb[:, n * TCH : (n + 1) * TCH],
                                    in_=o_ps,
                                    func=mybir.ActivationFunctionType.Identity,
                                )
                            else:
                                nc.vector.tensor_copy(
                                    out=o_sb[:, n * TCH : (n + 1) * TCH],
                                    in_=o_ps,
                                )
                            if tmode == 3:
                                dq = (
                                    nc.sync
                                    if (2 * tb + n) % 2 == 0
                                    else nc.gpsimd
                                )
                        else:
                            nc.vector.tensor_copy(
                                out=o_sb[:, n * TCH : (n + 1) * TCH], in_=o_ps
                            )
                        dq.dma_start(
                            out=out_d[
                                tj * TCH + tb * 128 : tj * TCH + (tb + 1) * 128,
                                n * TCH : (n + 1) * TCH,
                            ],
                            in_=o_sb[:, n * TCH : (n + 1) * TCH],
                        )

            prio_mode = os.environ.get("K_PRIO", "1") == "1"
            if interleave:
                for j in range(NTJ):
                    emit_qkv_chunk(j)
                    emit_attention(j)
            elif prio_mode:
                # emit qkv first (program order = dataflow order), but give
                # attention tj a priority window starting right after qkv
                # chunk tj, so the scheduler fills attention stalls with
                # later qkv chunks
                cp = []
                for j in range(NTJ):
                    emit_qkv_chunk(j)
                    cp.append(tc.cur_priority)
                if pre3:
                    # tj3's S/exp/mask precompute as mid-kernel filler
                    # (window right after qkv chunk 3); its PV/norm/proj
                    # run last as a dense pure-PE tail
                    cpa = None
                    for j in range(NTJ - 1):
                        off = tc.cur_priority - cp[j]
                        with tc.high_priority(offset=off):
                            emit_attention(j)
                        if j == NTJ - 3:
                            cpa = tc.cur_priority
                    off = tc.cur_priority - cp[NTJ - 1]
                    with tc.high_priority(offset=off):
                        emit_attention(NTJ - 1, phase="scores")
                    if os.environ.get("K_PV3_ILV", "0") == "1" and cpa:
                        off = tc.cur_priority - cpa
                        with tc.high_priority(offset=off):
                            emit_attention(NTJ - 1, phase="pv")
                    else:
                        emit_attention(NTJ - 1, phase="pv")
                else:
                    for j in range(NTJ):
                        off = tc.cur_priority - cp[j]
                        with tc.high_priority(offset=off):
                            emit_attention(j)
            else:
                for j in range(NTJ):
                    emit_qkv_chunk(j)
                for j in range(NTJ):
                    emit_attention(j)
    return nc


_NC_CACHE: dict = {}


def _get_nc(mask_mode: str, has_qkv_bias: bool):
    key = (mask_mode, has_qkv_bias)
    if key not in _NC_CACHE:
        _NC_CACHE[key] = _build(mask_mode, has_qkv_bias)
    return _NC_CACHE[key]


def classify_inputs(mask, qkv_b):
    m2 = np.asarray(mask).reshape(T, T)
    if np.array_equal(m2 != 0, np.tril(np.ones((T, T), dtype=bool))):
        mask_mode = "causal"
    elif np.all(m2 != 0):
        mask_mode = "none"
    else:
        mask_mode = "generic"
    has_qkv_bias = bool(np.any(np.asarray(qkv_b) != 0.0))
    return mask_mode, has_qkv_bias


def prepare_in_maps(x, mask, qkv_w, qkv_b, proj_w, proj_b):
    x = np.asarray(x, dtype=np.float32)
    qkv_w = np.asarray(qkv_w, dtype=np.float32)
    qkv_b = np.asarray(qkv_b, dtype=np.float32)
    proj_w = np.asarray(proj_w, dtype=np.float32)
    mask_mode, has_qkv_bias = classify_inputs(mask, qkv_b)
    m2 = np.asarray(mask).reshape(T, T)

    tri_np = np.triu(np.ones((128, 128))).astype(NP_BF16)
    qkv_dr = os.environ.get("K_QKV_DR", "1") == "1"
    proj_dr = qkv_dr and os.environ.get("K_PROJ_DR", "1") == "1"

    def split8(a):
        hi = a.astype(NP_FP8)
        lo = (a - hi.astype(np.float32)).astype(NP_FP8)
        return np.ascontiguousarray(hi), np.ascontiguousarray(lo)

    in_maps = []
    for b in range(B):
        xT_b = np.ascontiguousarray(x[b].T)
        if qkv_dr:
            xhi_b, xlo_b = split8(xT_b)
        else:
            xT_b = xT_b.astype(NP_BF16)
        for g in range(G):
            qs = qkv_w[:, g * VC : (g + 1) * VC]
            ks = qkv_w[:, D_MODEL + g * VC : D_MODEL + (g + 1) * VC]
            vs = qkv_w[:, 2 * D_MODEL + g * VC : 2 * D_MODEL + (g + 1) * VC]
            w_g = np.ascontiguousarray(np.concatenate([qs, ks, vs], axis=1))
            wp_g = proj_w[g * VC : (g + 1) * VC, :]
            if qkv_dr:
                # qkv weights pre-scaled so the fp8 lo-residual stays out of
                # the e4m3 subnormal flush zone; undone via exp scale (q,k)
                # and the proj weights (v)
                w_g = w_g * W_SCALE
                if proj_dr:
                    # proj weights also up-scaled for their own fp8 split;
                    # host divides the partials by W_SCALE**2
                    wp_g = wp_g * W_SCALE
                else:
                    wp_g = wp_g / W_SCALE
            im = {
                "wproj": np.ascontiguousarray(wp_g).astype(NP_BF16),
            }
            if qkv_dr:
                im["xhi"], im["xlo"] = xhi_b, xlo_b
                im["whi"], im["wlo"] = split8(w_g)
            if proj_dr:
                im["wphi"], im["wplo"] = split8(np.ascontiguousarray(wp_g))
            if not qkv_dr:
                im["xT"] = xT_b
                im["wqkv"] = w_g.astype(NP_BF16)
            if mask_mode == "causal":
                im["tri"] = tri_np
            if mask_mode == "generic":
                im["maskT"] = np.ascontiguousarray(
                    (m2 != 0).T.astype(NP_BF16)
                )
            if has_qkv_bias:
                b_scale = W_SCALE if qkv_dr else 1.0
                im["bqk"] = np.ascontiguousarray(
                    np.concatenate(
                        [qkv_b[g * VC : (g + 1) * VC],
                         qkv_b[D_MODEL + g * VC : D_MODEL + (g + 1) * VC]]
                    )
                    * b_scale
                ).astype(np.float32)
                im["bv"] = np.ascontiguousarray(
                    qkv_b[2 * D_MODEL + g * VC : 2 * D_MODEL + (g + 1) * VC]
                    * b_scale
                ).astype(np.float32)
            in_maps.append(im)
    return in_maps


def kernel(x, mask, qkv_w, qkv_b, proj_w, proj_b):
    proj_b = np.asarray(proj_b, dtype=np.float32)
    mask_mode, has_qkv_bias = classify_inputs(mask, qkv_b)
    nc = _get_nc(mask_mode, has_qkv_bias)
    in_maps = prepare_in_maps(x, mask, qkv_w, qkv_b, proj_w, proj_b)

    trace = bool(os.environ.get("KERNEL_TRACE"))
    res = run_bass_kernel_spmd(
        nc, in_maps, core_ids=list(range(B * G)), trace=trace
    )
    globals()["LAST_RESULT"] = res
    outs = [np.asarray(r["out"]).astype(np.float32) for r in res.results]

    qkv_dr = os.environ.get("K_QKV_DR", "1") == "1"
    proj_dr = qkv_dr and os.environ.get("K_PROJ_DR", "1") == "1"
    p_scale = 1.0 / (W_SCALE * W_SCALE) if proj_dr else 1.0
    final = np.empty((B, T, D_MODEL), dtype=np.float32)
    for b in range(B):
        acc = outs[b * G].copy()
        for g in range(1, G):
            acc += outs[b * G + g]
        final[b] = acc * p_scale + proj_b[None, :]
    return final

